# revision 31
# baseline (speedup 1.0000x reference)
"""Trainium2 Bass kernel for nn_MultiHeadAttention_88192858456426.

Reference (per batch b, C=512 channels, N=2048 tokens):
    qp = wq @ q + bq; kp = wk @ k + bk; vp = wv @ v + bv      # [C, N]
    S = qp^T kp  (no softmax);  out = (S @ vp^T)^T + q        # [C, N]

No softmax => the chain is linear and reassociates.  With G = k v^T:
    T  = kp vp^T = wk G wv^T + a x bv + bk x b    # a, b host vectors
    out = (T^T wq + I) q + (T^T bq) 1^T
All weight-side factors fold on the host: W1 = wk^T wq, u1 = wk^T bq,
hm = (wq^T a) x bv + (wq^T bk) x b + I.  The device computes
    G' = v k^T                    # [j2, j], 16-chunk accumulation
    AT = G wv^T                   # lhsT = G' slices, rhs = wv^T
    UT = W1^T AT  (+ hm via DVE)  # [i, c']
    ubias = AT^T u1 + w           # w host vector
    out = UT^T q + ubias 1^T
~84K PE cycles/core vs 360K for the direct form; no device transposes
(host supplies k,v as [N, C]).  Operands fp16, PSUM f32.

Every multi-tile tensor is host-packed into one [128, X] layout
(partition p holds row m*128+p of chunk m) so each rep needs only ~13
DMAs and 2 PE-side DMA waits instead of ~55/32 — DMA completion
latency and sem overhead are paid once per tensor, not per chunk.
Next-rep inputs are emitted ahead of this rep's output DMAs in the
in-order queues.  Output is fp16 (host casts back to f32).
Rel err ~5.9e-4 against the fp32 reference.
"""

import numpy as np
from contextlib import ExitStack

import concourse.bass as bass
import concourse.mybir as mybir
import concourse.tile as tile
from concourse import bacc
from concourse.bass_utils import run_bass_kernel_spmd

P = 128            # partitions
C = 512            # channels
N = 2048           # tokens
NB = 512           # n-block width (one PSUM bank of fp32)
CK = C // P        # 4 channel chunks
MCH = N // P       # 16 token chunks
NBK = N // NB      # 4 n-blocks

F32 = mybir.dt.float32
FP16 = mybir.dt.float16
ACT_IDENT = mybir.ActivationFunctionType.Identity

N_CORES = 8


def build_nc(reps=1, mode="fp16"):
    MDT = FP16
    nc = bacc.Bacc("TRN2", target_bir_lowering=False, debug=False,
                   num_devices=N_CORES)

    kT_d = nc.dram_tensor("kT2", [P, MCH * C], MDT,
                          kind="ExternalInput").ap()
    vT_d = nc.dram_tensor("vT2", [P, MCH * C], MDT,
                          kind="ExternalInput").ap()
    q_d = nc.dram_tensor("q2", [P, CK * N], MDT, kind="ExternalInput").ap()
    w1_d = nc.dram_tensor("w12", [P, CK * C], MDT,
                          kind="ExternalInput").ap()
    wv_d = nc.dram_tensor("wv2", [P, CK * C], MDT,
                          kind="ExternalInput").ap()
    hm_d = nc.dram_tensor("hm2", [P, CK * C], MDT,
                          kind="ExternalInput").ap()
    u1c_d = nc.dram_tensor("u1c", [P, CK], MDT, kind="ExternalInput").ap()
    wsb_d = nc.dram_tensor("wsb", [P, CK], F32, kind="ExternalInput").ap()
    oh_d = nc.dram_tensor("oh", [1, CK * CK], MDT, kind="ExternalInput").ap()
    o_d = nc.dram_tensor("o2", [P, NBK * CK * NB], MDT,
                         kind="ExternalOutput").ap()

    with ExitStack() as ctx:
        tc = ctx.enter_context(tile.TileContext(nc))
        consts = ctx.enter_context(tc.tile_pool(name="consts", bufs=1))
        wpool = ctx.enter_context(tc.tile_pool(name="wpool", bufs=1))
        kraw = ctx.enter_context(tc.tile_pool(name="kraw", bufs=2))
        vraw = ctx.enter_context(tc.tile_pool(name="vraw", bufs=2))
        qraw = ctx.enter_context(tc.tile_pool(name="qraw", bufs=2))
        gpool = ctx.enter_context(tc.tile_pool(name="gpool", bufs=1))
        atpool = ctx.enter_context(tc.tile_pool(name="atpool", bufs=1))
        utpool = ctx.enter_context(tc.tile_pool(name="utpool", bufs=1))
        ubpool = ctx.enter_context(tc.tile_pool(name="ubpool", bufs=1))
        opool = ctx.enter_context(tc.tile_pool(name="opool", bufs=5))
        ps_g = ctx.enter_context(tc.tile_pool(name="ps_g", bufs=4,
                                              space="PSUM"))
        ps_p = ctx.enter_context(tc.tile_pool(name="ps_p", bufs=2,
                                              space="PSUM"))
        ps_u = ctx.enter_context(tc.tile_pool(name="ps_u", bufs=1,
                                              space="PSUM"))

        def emit_inputs():
            """One DMA per packed tensor; next-rep inputs queue ahead of
            the current rep's output DMAs."""
            s = {}
            QW = MCH * C // 4
            t = kraw.tile([P, MCH * C], MDT, tag="kt", name="kt")
            for h in range(4):
                # quarter-granular so the first G matmuls start ~1.4us in
                # instead of waiting for the whole 2MB transfer
                nc.sync.dma_start(t[:, h * QW:(h + 1) * QW],
                                  kT_d[:, h * QW:(h + 1) * QW])
            s["kt"] = t
            t = vraw.tile([P, MCH * C], MDT, tag="vt", name="vt")
            for h in range(4):
                nc.scalar.dma_start(t[:, h * QW:(h + 1) * QW],
                                    vT_d[:, h * QW:(h + 1) * QW])
            s["vt"] = t
            t = wpool.tile([P, CK * C], MDT, tag="wv", name="wv")
            nc.scalar.dma_start(t[:], wv_d[:])
            s["wv"] = t
            t = wpool.tile([P, CK * C], MDT, tag="w1", name="w1")
            nc.sync.dma_start(t[:], w1_d[:])
            s["w1"] = t
            t = consts.tile([P, CK], MDT, tag="u1c", name="u1c")
            nc.sync.dma_start(t[:], u1c_d[:])
            s["u1c"] = t
            t = consts.tile([P, CK], F32, tag="wsb", name="wsb")
            nc.scalar.dma_start(t[:], wsb_d[:])
            s["wsb"] = t
            t = consts.tile([1, CK * CK], MDT, tag="oh", name="oh")
            nc.scalar.dma_start(t[:], oh_d[:])
            s["oh"] = t
            t = wpool.tile([P, CK * C], MDT, tag="hm", name="hm")
            nc.scalar.dma_start(t[:], hm_d[:])
            s["hm"] = t
            t = qraw.tile([P, CK * N], MDT, tag="q", name="q")
            nc.sync.dma_start(t[:], q_d[:])
            s["q"] = t
            return s

        cur = emit_inputs()
        for rep in range(reps):
            kt, vt, q = cur["kt"], cur["vt"], cur["q"]
            w1, wv, hm = cur["w1"], cur["wv"], cur["hm"]
            u1c, wsb, oh = cur["u1c"], cur["wsb"], cur["oh"]

            g_ps = [ps_g.tile([P, C], F32, tag="g_ps", name="g_ps")
                    for _ in range(CK)]

            # ---- G'[j2,j] = sum_m vT[m,j2] kT[m,j] over 16 m-chunks ----
            for m in range(MCH):
                for c in range(CK):
                    nc.tensor.matmul(
                        g_ps[c][:],
                        vt[:, m * C + c * P:m * C + (c + 1) * P],
                        kt[:, m * C:(m + 1) * C],
                        start=(m == 0), stop=(m == MCH - 1))

            g_sb = []
            for c in range(CK):
                t = gpool.tile([P, C], MDT, tag=f"g{c}", name=f"g{c}")
                if c % 2 == 0:
                    nc.scalar.copy(t[:], g_ps[c][:])
                else:
                    nc.vector.tensor_copy(t[:], g_ps[c][:])
                g_sb.append(t)

            # ---- AT[j,c'] = sum_j2 G'[j2,j] wvT[j2,c']  (= G wv^T) ----
            at_sb = []
            for j in range(CK):
                ps = ps_p.tile([P, C], F32, tag="ps_p", name="ps_p")
                for j2 in range(CK):
                    nc.tensor.matmul(ps[:],
                                     g_sb[j2][:, j * P:(j + 1) * P],
                                     wv[:, j2 * C:(j2 + 1) * C],
                                     start=(j2 == 0), stop=(j2 == CK - 1))
                t = atpool.tile([P, C], MDT, tag=f"at{j}", name=f"at{j}")
                if j % 2 == 0:
                    nc.scalar.copy(t[:], ps[:])
                else:
                    nc.vector.tensor_copy(t[:], ps[:])
                at_sb.append(t)

            # ---- ubias[c'] = sum_j u1[j] AT[j,c'] as a [1, C] row:
            # lhsT is a single u1 column so LDWEIGHTS is ~free ----
            ubr_ps = ps_u.tile([1, C], F32, tag="ubr_ps", name="ubr_ps")
            for j in range(CK):
                nc.tensor.matmul(ubr_ps[:], u1c[:, j:j + 1], at_sb[j][:],
                                 start=(j == 0), stop=(j == CK - 1))
            ubr = ubpool.tile([1, C], MDT, tag="ubr", name="ubr")
            nc.scalar.copy(ubr[:], ubr_ps[:])

            # ---- UT[i,c'] = sum_j W1[j,i] AT[j,c'] + hm[i,c'] ----
            # (the ubr PSUM->SBUF copy drains while these stream)
            ut_sb = []
            for i in range(CK):
                ps = ps_p.tile([P, C], F32, tag="ps_p", name="ps_p")
                for j in range(CK):
                    nc.tensor.matmul(
                        ps[:],
                        w1[:, j * C + i * P:j * C + (i + 1) * P],
                        at_sb[j][:],
                        start=(j == 0), stop=(j == CK - 1))
                ut = utpool.tile([P, C], MDT, tag=f"ut{i}", name=f"ut{i}")
                nc.vector.tensor_add(ut[:], ps[:],
                                     hm[:, i * C:(i + 1) * C])
                ut_sb.append(ut)

            # transpose the ubias row to [P, CK] columns with one-hot
            # rhs matmuls (K=1, Nf=4: ~free), then add the host vector
            ub_ps = ps_u.tile([P, CK], F32, tag="ub_ps", name="ub_ps")
            for j in range(CK):
                nc.tensor.matmul(ub_ps[:], ubr[:, j * P:(j + 1) * P],
                                 oh[:, CK * j:CK * (j + 1)],
                                 start=(j == 0), stop=(j == CK - 1))
            ubias = ubpool.tile([P, CK], F32, tag="ubias", name="ubias")
            nc.vector.tensor_add(ubias[:], ub_ps[:], wsb[:])

            # next rep's inputs are emitted (and queued) ahead of this
            # rep's output DMAs so the next G-loop never waits on them
            if rep + 1 < reps:
                cur = emit_inputs()

            # ---- out[c',n] = sum_i UT[i,c'] q[i,n] + ubias[c'],
            # packed per n-block: one output DMA per nb ----
            for nb in range(NBK):
                o_sb = opool.tile([P, CK * NB], MDT, tag="o", name="o")
                for cp in range(CK):
                    ps = ps_g.tile([P, NB], F32, tag="g_ps", name="g_ps")
                    for i in range(CK):
                        nc.tensor.matmul(
                            ps[:],
                            ut_sb[i][:, cp * P:(cp + 1) * P],
                            q[:, i * N + nb * NB:i * N + (nb + 1) * NB],
                            start=(i == 0), stop=(i == CK - 1))
                    nc.scalar.activation(o_sb[:, cp * NB:(cp + 1) * NB],
                                         ps[:], ACT_IDENT,
                                         bias=ubias[:, cp:cp + 1])
                eng = nc.sync if nb % 2 == 0 else nc.scalar
                eng.dma_start(o_d[:, nb * CK * NB:(nb + 1) * CK * NB],
                              o_sb[:])

    nc.finalize()
    return nc


_CACHE = {}


MODE = "fp16"


def _get_nc():
    if "nc" not in _CACHE:
        _CACHE["nc"] = build_nc(mode=MODE)
    return _CACHE["nc"]


def _pack(x, nchunk):
    """[nchunk*P, X] -> [P, nchunk*X]: partition p holds row c*P+p of
    chunk c at columns [c*X, (c+1)*X)."""
    n, xw = x.shape
    assert n == nchunk * P
    return np.ascontiguousarray(
        x.reshape(nchunk, P, xw).transpose(1, 0, 2).reshape(P, nchunk * xw))


def _in_maps(q, k, v, wq, bq, wk, bk, wv, bv, mode=None):
    f32 = lambda x: np.ascontiguousarray(np.asarray(x), dtype=np.float32)
    h16 = lambda x: np.ascontiguousarray(np.asarray(x), dtype=np.float16)
    q16 = np.asarray(q, dtype=np.float16)
    k64 = np.asarray(k, dtype=np.float64)
    v64 = np.asarray(v, dtype=np.float64)
    wqf, wkf, wvf = (np.asarray(w, dtype=np.float64)
                     for w in (wq, wk, wv))
    bqf, bkf, bvf = (np.asarray(x, dtype=np.float64)
                     for x in (bq, bk, bv))
    kT = np.swapaxes(k64, 1, 2).astype(np.float16)      # [B, N, C]
    vT = np.swapaxes(v64, 1, 2).astype(np.float16)
    w1 = (wkf.T @ wqf).astype(np.float16)
    wvT = wvf.T.astype(np.float16)
    u1c = h16((wkf.T @ bqf).reshape(CK, P).T)

    rk = k64.sum(2)                                     # [B, C]
    rv = v64.sum(2)
    a = rk @ wkf.T + N * bkf[None, :]                   # [B, C]
    bvec = rv @ wvf.T                                   # [B, C]
    a2 = a @ wqf                                        # [B, C]
    c2 = wqf.T @ bkf                                    # [C]
    s1 = a @ bqf                                        # [B]
    s2 = float(bqf @ bkf)
    w = s1[:, None] * bvf[None, :] + s2 * bvec          # [B, C]
    eye = np.eye(C)
    hm = (a2[:, :, None] * bvf[None, None, :]
          + c2[None, :, None] * bvec[:, None, :]
          + eye[None]).astype(np.float16)               # [B, C, C]

    oh = np.ascontiguousarray(
        np.eye(CK, dtype=np.float16).reshape(1, CK * CK))
    w1p = _pack(w1, CK)
    wvp = _pack(wvT, CK)
    return [
        {"kT2": _pack(kT[i], MCH), "vT2": _pack(vT[i], MCH),
         "q2": _pack(q16[i], CK),
         "w12": w1p, "wv2": wvp, "hm2": _pack(hm[i], CK), "u1c": u1c,
         "wsb": f32(w[i].reshape(CK, P).T), "oh": oh}
        for i in range(N_CORES)
    ]


def _unpack_out(o2):
    """[P, NBK*CK*NB] -> [C, N]: o2[p, nb*CK*NB + cp*NB + j] =
    out[cp*P + p, nb*NB + j]."""
    return np.ascontiguousarray(
        o2.reshape(P, NBK, CK, NB).transpose(2, 0, 1, 3).reshape(C, N))


def run(inputs, **spmd_kwargs):
    """Run on hardware; returns (output [B,C,N], BassKernelResults)."""
    nc = _get_nc()
    maps = _in_maps(**inputs)
    res = run_bass_kernel_spmd(nc, maps, list(range(N_CORES)), **spmd_kwargs)
    out = np.stack([_unpack_out(res.results[i]["o2"])
                    for i in range(N_CORES)], axis=0).astype(np.float32)
    return out, res


def kernel(q, k, v, wq, bq, wk, bk, wv, bv):
    out, _ = run(dict(q=q, k=k, v=v, wq=wq, bq=bq, wk=wk, bk=bk,
                      wv=wv, bv=bv))
    return out
